# revision 12
# baseline (speedup 1.0000x reference)
"""Trainium2 Bass kernel for nn_EntityEncoder (GNN message passing encoder).

Data-parallel over batch B=1024 across 8 NeuronCores (128 batches each).
Device strategy:
  - ent_W row gathers (6400/core from the 500K x 128 table) + h/t/r lookups:
    per-partition indirect DMA gathers (order-preserving, exact f32).
  - rel_W K-mean gathers (51200 rows/core from the 1000-row table):
    TensorEngine matmul against a host-built count matrix (fp8 counts/16,
    exact) -- avoids the slow per-row Q7 descriptor path.
  - GCN bmm: per-batch PE matmuls with host-pre-transposed adjacency;
    attention softmax weights folded into the adjacency columns.
  - BatchNorm1 stats: on-device AllReduce of per-core partials. BatchNorm2
    affects only channel 0 of the output; its two global scalars are applied
    on the host.
"""
import sys

sys.path.insert(0, "/opt/trn_rl_repo")

import numpy as np
import ml_dtypes

B, N, K, D = 1024, 50, 8, 128
NUM_ENTS, NUM_RELS = 500000, 1000
NC = 8
BL = B // NC          # 128 batches per core
T = BL * N            # 6400 subg tokens per core
EPS = 1e-5
NB_CALLS = T // 128   # 50 indirect calls for neb
CH = 640              # token chunk for rel/Z matmuls (5 tiles of 128)
NCHUNK = T // CH      # 10
SC = 800              # score chunk (16 batches of 50 tokens)
NSC = T // SC         # 8

_cache = {}


def _build_program():
    import concourse.bass as bass
    import concourse.tile as tile
    from concourse import mybir, bacc

    f32 = mybir.dt.float32
    bf16 = mybir.dt.bfloat16
    fp8 = mybir.dt.float8e4
    i32 = mybir.dt.int32
    Alu = mybir.AluOpType
    Act = mybir.ActivationFunctionType

    nc = bacc.Bacc(None, target_bir_lowering=False, num_devices=NC)

    # ---- DRAM I/O ----
    ent = nc.dram_tensor("ent", [NUM_ENTS, D], f32, kind="ExternalInput")
    rel = nc.dram_tensor("rel", [NUM_RELS, D], f32, kind="ExternalInput")
    neb_off = nc.dram_tensor("neb_off", [128, NB_CALLS], i32, kind="ExternalInput")
    h_off = nc.dram_tensor("h_off", [128, 1], i32, kind="ExternalInput")
    t_off = nc.dram_tensor("t_off", [128, 1], i32, kind="ExternalInput")
    r_off = nc.dram_tensor("r_off", [128, 1], i32, kind="ExternalInput")
    c2 = nc.dram_tensor("c2", [NCHUNK, 128, 8 * CH], fp8, kind="ExternalInput")
    relw16 = nc.dram_tensor("relw16", [8, 128, D], bf16, kind="ExternalInput")
    adjts_d = nc.dram_tensor("adjts", [50, BL * 51], f32, kind="ExternalInput")
    adjt0_d = nc.dram_tensor("adjt0", [1, BL * 51], f32, kind="ExternalInput")
    a0 = nc.dram_tensor("a0", [128, BL], f32, kind="ExternalInput")
    att_wt = nc.dram_tensor("att_wt", [D, D], f32, kind="ExternalInput")
    att_w = nc.dram_tensor("att_w", [D, D], f32, kind="ExternalInput")
    gcn_wt = nc.dram_tensor("gcn_wt", [D, D], f32, kind="ExternalInput")
    gcn_wth = nc.dram_tensor("gcn_wth", [D, D], f32, kind="ExternalInput")
    att_b = nc.dram_tensor("att_b", [D, 1], f32, kind="ExternalInput")
    gcn_b = nc.dram_tensor("gcn_b", [D, 1], f32, kind="ExternalInput")
    gam = nc.dram_tensor("gam", [51, 1], f32, kind="ExternalInput")
    bet = nc.dram_tensor("bet", [51, 1], f32, kind="ExternalInput")

    h_out = nc.dram_tensor("h_out", [128, D], f32, kind="ExternalOutput")
    t_out = nc.dram_tensor("t_out", [128, D], f32, kind="ExternalOutput")
    r_out = nc.dram_tensor("r_out", [128, D], f32, kind="ExternalOutput")
    y2_out = nc.dram_tensor("y2_out", [128, D], f32, kind="ExternalOutput")
    s2_out = nc.dram_tensor("s2_out", [1, 2], f32, kind="ExternalOutput")

    with tile.TileContext(nc) as tc:
        with (
            tc.tile_pool(name="big", bufs=1) as big,
            tc.tile_pool(name="med", bufs=1) as med,
            tc.tile_pool(name="ring", bufs=2) as ring,
            tc.tile_pool(name="tiny", bufs=1) as tiny,
            tc.tile_pool(name="psA", bufs=2, space="PSUM") as psA,
            tc.tile_pool(name="psB", bufs=2, space="PSUM") as psB,
            tc.tile_pool(name="psC", bufs=2, space="PSUM") as psC,
            tc.tile_pool(name="dram", bufs=1, space="DRAM") as dpool,
        ):
            # ---- constants ----
            def scaled_ident(val, nm):
                t_ = tiny.tile([128, 128], f32, tag=nm)
                nc.gpsimd.memset(t_[:], 0.0)
                nc.gpsimd.affine_select(
                    out=t_[:], in_=t_[:], compare_op=Alu.not_equal, fill=val,
                    base=0, pattern=[[-1, 128]], channel_multiplier=1)
                return t_

            ident = scaled_ident(1.0, "id1")
            onescol = tiny.tile([128, 1], f32, tag="ones")
            nc.vector.memset(onescol[:], 1.0)

            def preload(dram_t, shape, nm, dt_=f32):
                t_ = tiny.tile(shape, dt_, tag=nm)
                nc.sync.dma_start(out=t_[:], in_=dram_t[:])
                return t_

            attwt_sb = preload(att_wt, [D, D], "attwt")
            attw_sb = preload(att_w, [D, D], "attw")
            gcnwt_sb = preload(gcn_wt, [D, D], "gcnwt")
            gcnwth_sb = preload(gcn_wth, [D, D], "gcnwth")
            attb_sb = preload(att_b, [D, 1], "attb")
            gcnb_sb = preload(gcn_b, [D, 1], "gcnb")
            gam_sb = preload(gam, [51, 1], "gam")
            bet_sb = preload(bet, [51, 1], "bet")
            a0_sb = preload(a0, [128, BL], "a0")
            nebo_sb = preload(neb_off, [128, NB_CALLS], "nebo", i32)
            ho_sb = preload(h_off, [128, 1], "ho", i32)
            to_sb = preload(t_off, [128, 1], "to", i32)
            ro_sb = preload(r_off, [128, 1], "ro", i32)
            relw_sb = tiny.tile([128, 8, D], bf16, tag="relw")
            for s in range(8):
                nc.sync.dma_start(out=relw_sb[:, s, :], in_=relw16[s, :, :])
            adjts_sb = med.tile([50, BL * 51], f32, tag="adjts")
            nc.sync.dma_start(out=adjts_sb[:], in_=adjts_d[:])
            adjt0_sb = med.tile([1, BL * 51], f32, tag="adjt0")
            nc.sync.dma_start(out=adjt0_sb[:], in_=adjt0_d[:])

            # ---- h/t/r gathers (token-major, exact f32) ----
            def igather(dst, table, offs):
                nc.gpsimd.indirect_dma_start(
                    out=dst, out_offset=None, in_=table[:],
                    in_offset=bass.IndirectOffsetOnAxis(ap=offs, axis=0))

            hg = med.tile([128, D], f32, tag="hg")
            tg = med.tile([128, D], f32, tag="tg")
            rg = med.tile([128, D], f32, tag="rg")
            igather(hg[:], ent, ho_sb[:, 0:1])
            igather(tg[:], ent, to_sb[:, 0:1])
            igather(rg[:], rel, ro_sb[:, 0:1])
            nc.sync.dma_start(out=h_out[:], in_=hg[:])
            nc.sync.dma_start(out=t_out[:], in_=tg[:])
            nc.sync.dma_start(out=r_out[:], in_=rg[:])

            # S = (h+t+r)/3 D-major via 3 accumulating scaled transposes
            s_ps = psC.tile([128, 128], f32, tag="sc", space="PSUM")
            nc.tensor.matmul(out=s_ps[:], lhsT=hg[:], rhs=ident[:],
                             is_transpose=True, start=True, stop=False)
            nc.tensor.matmul(out=s_ps[:], lhsT=tg[:], rhs=ident[:],
                             is_transpose=True, start=False, stop=False)
            nc.tensor.matmul(out=s_ps[:], lhsT=rg[:], rhs=ident[:],
                             is_transpose=True, start=False, stop=True)
            s_sb = med.tile([128, 128], f32, tag="ssb")
            nc.vector.tensor_scalar_mul(out=s_sb[:], in0=s_ps[:],
                                        scalar1=1.0 / 3.0)

            # ---- neb gather: 50 indirect calls, token-major ----
            nebg = big.tile([128, NB_CALLS, D], f32, tag="bb")
            for j in range(NB_CALLS):
                igather(nebg[:, j, :], ent, nebo_sb[:, j:j + 1])

            # ---- subgT = 0.5*neb_eT + (1/16)*sum_k rel_W[nebr] ----
            subgT = big.tile([128, T], f32, tag="subgT")
            for ch in range(NCHUNK):
                c2f8 = ring.tile([128, 8 * CH], fp8, tag="c2f8")
                nc.sync.dma_start(out=c2f8[:], in_=c2[ch, :, :])
                rel_ps = psA.tile([128, CH], f32, tag="mm", space="PSUM")
                for s in range(8):
                    c2bf = ring.tile([128, CH], bf16, tag="c2bf")
                    nc.vector.tensor_copy(
                        out=c2bf[:], in_=c2f8[:, s * CH:(s + 1) * CH])
                    nc.tensor.matmul(out=rel_ps[:, 0:512],
                                     lhsT=relw_sb[:, s, :], rhs=c2bf[:, 0:512],
                                     start=(s == 0), stop=(s == 7))
                    nc.tensor.matmul(out=rel_ps[:, 512:CH],
                                     lhsT=relw_sb[:, s, :], rhs=c2bf[:, 512:CH],
                                     start=(s == 0), stop=(s == 7))
                relc_sb = ring.tile([128, CH], f32, tag="relc")
                nc.vector.tensor_copy(out=relc_sb[:], in_=rel_ps[:])
                for jj in range(CH // 128):
                    j = ch * (CH // 128) + jj
                    tr_ps = psB.tile([128, 128], f32, tag="tr", space="PSUM")
                    nc.tensor.matmul(out=tr_ps[:], lhsT=nebg[:, j, :],
                                     rhs=ident[:], is_transpose=True,
                                     start=True, stop=True)
                    nc.vector.tensor_tensor(
                        out=subgT[:, j * 128:(j + 1) * 128],
                        in0=tr_ps[:], in1=relc_sb[:, jj * 128:(jj + 1) * 128],
                        op=Alu.add)

            # ---- Q, U, c ----
            q_ps = psA.tile([128, 128], f32, tag="mm", space="PSUM")
            nc.tensor.matmul(out=q_ps[:], lhsT=attwt_sb[:], rhs=s_sb[:],
                             start=True, stop=True)
            q_sb = med.tile([128, 128], f32, tag="qsb")
            nc.scalar.activation(out=q_sb[:], in_=q_ps[:], func=Act.Identity,
                                 bias=attb_sb[:, 0:1], scale=1.0)
            u_ps = psA.tile([128, 128], f32, tag="mm", space="PSUM")
            nc.tensor.matmul(out=u_ps[:], lhsT=attw_sb[:], rhs=q_sb[:],
                             start=True, stop=True)
            u_sb = med.tile([128, 128], f32, tag="usb")
            nc.vector.tensor_scalar_mul(out=u_sb[:], in0=u_ps[:], scalar1=0.5)
            c_ps = psC.tile([1, 128], f32, tag="sc", space="PSUM")
            nc.tensor.matmul(out=c_ps[:], lhsT=attb_sb[:, 0:1], rhs=q_sb[:],
                             start=True, stop=True)
            c_sb = tiny.tile([1, 128], f32, tag="csb")
            nc.vector.tensor_copy(out=c_sb[:], in_=c_ps[:])
            ct_sb = tiny.tile([128, 1], f32, tag="ctsb")
            nc.sync.dma_start(out=ct_sb[:], in_=c_sb[:])

            # ---- attention scores ----
            scores_dr = dpool.tile([1, T], f32)
            for ch in range(NSC):           # 800 tokens = 16 batches
                prod = ring.tile([128, SC], f32, tag="prod")
                nc.vector.tensor_tensor(
                    out=prod[:].rearrange("p (b n) -> p b n", n=N),
                    in0=subgT[:, ch * SC:(ch + 1) * SC].rearrange(
                        "p (b n) -> p b n", n=N),
                    in1=u_sb[:, ch * 16:(ch + 1) * 16].rearrange(
                        "p (b one) -> p b one", one=1).to_broadcast([128, 16, N]),
                    op=Alu.mult)
                sc_sb = med.tile([1, SC], f32, tag="scsb")
                for (o0, o1) in ((0, 512), (512, SC)):
                    sc_ps = psC.tile([1, 512], f32, tag="sc", space="PSUM")
                    nc.tensor.matmul(out=sc_ps[:, :o1 - o0], lhsT=onescol[:],
                                     rhs=prod[:, o0:o1], start=True, stop=True)
                    nc.vector.tensor_copy(out=sc_sb[:, o0:o1],
                                          in_=sc_ps[:, :o1 - o0])
                nc.sync.dma_start(out=scores_dr[:, ch * SC:(ch + 1) * SC],
                                  in_=sc_sb[:])
            scoresb = med.tile([128, N], f32, tag="scoresb")
            nc.sync.dma_start(out=scoresb[:], in_=scores_dr[:])
            nc.vector.tensor_scalar_add(out=scoresb[:], in0=scoresb[:],
                                        scalar1=ct_sb[:, 0:1])
            nc.scalar.activation(out=scoresb[:], in_=scoresb[:], func=Act.Lrelu,
                                 alpha=0.01)
            esum = tiny.tile([128, 1], f32, tag="esum")
            nc.scalar.activation(out=scoresb[:], in_=scoresb[:], func=Act.Exp,
                                 accum_out=esum[:, 0:1])
            einv = tiny.tile([128, 1], f32, tag="einv")
            nc.vector.reciprocal(out=einv[:], in_=esum[:])
            nc.vector.tensor_scalar_mul(out=scoresb[:], in0=scoresb[:],
                                        scalar1=einv[:, 0:1])

            # att weights transposed [50, 128]; fold into adjts columns
            att_ps = psB.tile([N, 128], f32, tag="tr", space="PSUM")
            nc.tensor.matmul(out=att_ps[:], lhsT=scoresb[:], rhs=ident[:],
                             is_transpose=True, start=True, stop=True)
            atts = med.tile([N, 128], f32, tag="atts")
            nc.vector.tensor_copy(out=atts[:], in_=att_ps[:])
            nc.vector.tensor_tensor(
                out=adjts_sb[:].rearrange("p (b i) -> p b i", b=BL),
                in0=adjts_sb[:].rearrange("p (b i) -> p b i", b=BL),
                in1=atts[:].rearrange("p (b one) -> p b one", one=1
                                      ).to_broadcast([N, BL, 51]),
                op=Alu.mult)

            # ---- Z = gcn_W @ [subgT | S] + gcn_b ----
            z_sb = big.tile([128, T + 128], f32, tag="bb")  # reuses nebg slot
            for ch in range(NCHUNK):
                z_ps = psA.tile([128, CH], f32, tag="mm", space="PSUM")
                nc.tensor.matmul(out=z_ps[:, 0:512], lhsT=gcnwth_sb[:],
                                 rhs=subgT[:, ch * CH:ch * CH + 512],
                                 start=True, stop=True)
                nc.tensor.matmul(out=z_ps[:, 512:CH], lhsT=gcnwth_sb[:],
                                 rhs=subgT[:, ch * CH + 512:(ch + 1) * CH],
                                 start=True, stop=True)
                nc.scalar.activation(out=z_sb[:, ch * CH:(ch + 1) * CH],
                                     in_=z_ps[:], func=Act.Identity,
                                     bias=gcnb_sb[:, 0:1], scale=1.0)
            zs_ps = psA.tile([128, 128], f32, tag="mm", space="PSUM")
            nc.tensor.matmul(out=zs_ps[:], lhsT=gcnwt_sb[:], rhs=s_sb[:],
                             start=True, stop=True)
            nc.scalar.activation(out=z_sb[:, T:T + 128], in_=zs_ps[:],
                                 func=Act.Identity, bias=gcnb_sb[:, 0:1],
                                 scale=1.0)

            # WS_T token-major -> DRAM (for base-0 row access later)
            wst_ps = psB.tile([128, 128], f32, tag="tr", space="PSUM")
            nc.tensor.matmul(out=wst_ps[:], lhsT=z_sb[:, T:T + 128],
                             rhs=ident[:], is_transpose=True,
                             start=True, stop=True)
            wst = med.tile([128, 128], f32, tag="wst")
            nc.vector.tensor_copy(out=wst[:], in_=wst_ps[:])
            wst_dr = dpool.tile([128, 128], f32)
            nc.sync.dma_start(out=wst_dr[:], in_=wst[:])

            # ---- bmm1 + relu + stats (pairs at partitions 0 / 64) ----
            y1 = big.tile([128, (BL // 2) * 128], f32, tag="y1")
            sum1 = tiny.tile([128, 16], f32, tag="sum1")
            sum2 = tiny.tile([128, 16], f32, tag="sum2")
            for g8 in range(BL // 8):
                wst0 = med.tile([1, 1024], f32, tag="wst0")
                nc.sync.dma_start(out=wst0[:],
                                  in_=wst_dr[g8 * 8:(g8 + 1) * 8, :])
                zt_sb = med.tile([N, 1024], f32, tag="zt")
                for half in range(2):
                    ztq_ps = psB.tile([N, 512], f32, tag="tr", space="PSUM")
                    for bi in range(4):
                        b = g8 * 8 + half * 4 + bi
                        nc.tensor.matmul(
                            out=ztq_ps[:, bi * 128:(bi + 1) * 128],
                            lhsT=z_sb[:, b * N:(b + 1) * N], rhs=ident[:],
                            is_transpose=True, start=True, stop=True)
                    nc.vector.tensor_copy(
                        out=zt_sb[:, half * 512:(half + 1) * 512], in_=ztq_ps[:])
                y1_ps = psA.tile([128, 512], f32, tag="mm", space="PSUM")
                for bi in range(8):
                    b = g8 * 8 + bi
                    po = (b % 2) * 64
                    co = (bi // 2) * 128
                    nc.tensor.matmul(
                        out=y1_ps[po:po + 51, co:co + 128],
                        lhsT=adjts_sb[:, b * 51:(b + 1) * 51],
                        rhs=zt_sb[:, bi * 128:(bi + 1) * 128],
                        start=True, stop=False)
                    nc.tensor.matmul(
                        out=y1_ps[po:po + 51, co:co + 128],
                        lhsT=adjt0_sb[0:1, b * 51:(b + 1) * 51],
                        rhs=wst0[0:1, bi * 128:(bi + 1) * 128],
                        start=False, stop=True)
                nc.scalar.activation(out=y1[:, g8 * 512:(g8 + 1) * 512],
                                     in_=y1_ps[:], func=Act.Relu,
                                     accum_out=sum1[:, g8:g8 + 1])
                sqbuf = med.tile([128, 512], f32, tag="sq")
                nc.scalar.activation(out=sqbuf[:],
                                     in_=y1[:, g8 * 512:(g8 + 1) * 512],
                                     func=Act.Square,
                                     accum_out=sum2[:, g8:g8 + 1])

            # ---- BN1 stats fold + AllReduce ----
            Ax = mybir.AxisListType
            st = tiny.tile([128, 2], f32, tag="st")
            nc.vector.tensor_reduce(out=st[:, 0:1], in_=sum1[:], axis=Ax.X,
                                    op=Alu.add)
            nc.vector.tensor_reduce(out=st[:, 1:2], in_=sum2[:], axis=Ax.X,
                                    op=Alu.add)
            sthi = tiny.tile([51, 2], f32, tag="sthi")
            nc.vector.tensor_copy(out=sthi[:], in_=st[64:115, :])
            stf = tiny.tile([51, 2], f32, tag="stf")
            nc.vector.tensor_tensor(out=stf[:], in0=st[0:51, :],
                                    in1=sthi[:], op=Alu.add)
            bnc_in = dpool.tile([51, 2], f32)
            bnc_out = dpool.tile([51, 2], f32)
            nc.gpsimd.dma_start(out=bnc_in[:], in_=stf[:])
            nc.gpsimd.collective_compute(
                "AllReduce", Alu.add, replica_groups=[list(range(NC))],
                ins=[bnc_in.opt()], outs=[bnc_out.opt()])
            stg = tiny.tile([51, 2], f32, tag="stg")
            nc.gpsimd.dma_start(out=stg[:], in_=bnc_out[:])

            inv_n = 1.0 / (B * 128.0)
            mcol = tiny.tile([51, 1], f32, tag="mcol")
            nc.vector.tensor_scalar_mul(out=mcol[:], in0=stg[:, 0:1],
                                        scalar1=inv_n)
            m2 = tiny.tile([51, 1], f32, tag="m2")
            nc.vector.tensor_mul(m2[:], mcol[:], mcol[:])
            vcol = tiny.tile([51, 1], f32, tag="vcol")
            nc.vector.tensor_scalar_mul(out=vcol[:], in0=stg[:, 1:2],
                                        scalar1=inv_n)
            nc.vector.tensor_sub(vcol[:], vcol[:], m2[:])
            epst = tiny.tile([51, 1], f32, tag="epst")
            nc.vector.memset(epst[:], EPS)
            sd = tiny.tile([51, 1], f32, tag="sd")
            nc.scalar.activation(out=sd[:], in_=vcol[:], func=Act.Sqrt,
                                 bias=epst[:, 0:1])
            rstd = tiny.tile([51, 1], f32, tag="rstd")
            nc.vector.reciprocal(out=rstd[:], in_=sd[:])
            alpha = tiny.tile([51, 1], f32, tag="alpha")
            nc.vector.tensor_mul(alpha[:], gam_sb[:], rstd[:])
            ccol = tiny.tile([51, 1], f32, tag="ccol")
            nc.vector.tensor_mul(ccol[:], mcol[:], alpha[:])
            nc.vector.tensor_sub(ccol[:], bet_sb[:], ccol[:])
            alpha2 = tiny.tile([128, 1], f32, tag="alpha2")
            nc.vector.memset(alpha2[:], 0.0)
            nc.vector.tensor_copy(out=alpha2[0:51, :], in_=alpha[:])
            nc.vector.tensor_copy(out=alpha2[64:115, :], in_=alpha[:])
            ccol2 = tiny.tile([128, 1], f32, tag="ccol2")
            nc.vector.memset(ccol2[:], 0.0)
            nc.vector.tensor_copy(out=ccol2[0:51, :], in_=ccol[:])
            nc.vector.tensor_copy(out=ccol2[64:115, :], in_=ccol[:])

            nc.vector.tensor_scalar(y1[:], y1[:], alpha2[:, 0:1],
                                    ccol2[:, 0:1], Alu.mult, Alu.add)

            # ---- bmm2 row 0 (column-major out) + relu + stats2 ----
            y2t_ps = psB.tile([128, 128], f32, tag="tr", space="PSUM")
            for b in range(BL):
                po = (b % 2) * 64
                co = (b // 2) * 128
                nc.tensor.matmul(
                    out=y2t_ps[:, b:b + 1],
                    lhsT=y1[po:po + 51, co:co + 128],
                    rhs=a0_sb[po:po + 51, b:b + 1],
                    start=True, stop=True)
            p1 = tiny.tile([128, 1], f32, tag="p1")
            y2t_sb = med.tile([128, 128], f32, tag="y2t")
            nc.scalar.activation(out=y2t_sb[:], in_=y2t_ps[:], func=Act.Relu,
                                 accum_out=p1[:, 0:1])
            p2 = tiny.tile([128, 1], f32, tag="p2")
            y2sq = med.tile([128, 128], f32, tag="y2sq")
            nc.scalar.activation(out=y2sq[:], in_=y2t_sb[:], func=Act.Square,
                                 accum_out=p2[:, 0:1])
            # transpose to token-major for output
            y2m_ps = psC.tile([128, 128], f32, tag="sc", space="PSUM")
            nc.tensor.matmul(out=y2m_ps[:], lhsT=y2t_sb[:], rhs=ident[:],
                             is_transpose=True, start=True, stop=True)
            y2m_sb = med.tile([128, 128], f32, tag="y2m")
            nc.vector.tensor_copy(out=y2m_sb[:], in_=y2m_ps[:])
            nc.sync.dma_start(out=y2_out[:], in_=y2m_sb[:])

            pc = tiny.tile([128, 2], f32, tag="pc")
            nc.vector.tensor_copy(out=pc[:, 0:1], in_=p1[:])
            nc.vector.tensor_copy(out=pc[:, 1:2], in_=p2[:])
            s2_ps = psC.tile([1, 2], f32, tag="sc", space="PSUM")
            nc.tensor.matmul(out=s2_ps[:], lhsT=onescol[:], rhs=pc[:],
                             start=True, stop=True)
            s2_sb = tiny.tile([1, 2], f32, tag="s2sb")
            nc.vector.tensor_copy(out=s2_sb[:], in_=s2_ps[:])
            nc.sync.dma_start(out=s2_out[:], in_=s2_sb[:])

    nc.finalize()
    return nc


def _prep_core(ci, hrt, neb, nebr, adj):
    b0, b1 = ci * BL, (ci + 1) * BL
    hrt_l = np.asarray(hrt[b0:b1], np.int64)
    neb_l = np.asarray(neb[b0:b1], np.int64)
    nebr_l = np.asarray(nebr[b0:b1], np.int64)
    adj_l = np.asarray(adj[b0:b1], np.float32)

    neb_flat = neb_l.reshape(T)
    neb_off = np.ascontiguousarray(neb_flat.reshape(NB_CALLS, 128).T).astype(np.int32)

    tok = np.repeat(np.arange(T, dtype=np.int64), K)
    rr = nebr_l.reshape(T * K)
    cnt = np.zeros((1024, T), np.float32)
    np.add.at(cnt, (rr, tok), 1.0)
    cnt *= (1.0 / 8.0)
    c2r = cnt.reshape(8, 128, NCHUNK, CH).transpose(2, 1, 0, 3).reshape(
        NCHUNK, 128, 8 * CH)
    c2r = np.ascontiguousarray(c2r).astype(ml_dtypes.float8_e4m3)

    adjt_full = adj_l.transpose(2, 0, 1)            # [j, b, i]
    adjts = np.ascontiguousarray(adjt_full[1:].reshape(50, BL * 51))
    adjt0 = np.ascontiguousarray(adjt_full[0:1].reshape(1, BL * 51))
    a0 = np.zeros((128, BL), np.float32)
    a0[0:51] = adj_l[:, 0, :].T
    a0[64:115] = adj_l[:, 0, :].T

    return {
        "neb_off": neb_off,
        "h_off": hrt_l[:, 0:1].astype(np.int32).copy(),
        "t_off": hrt_l[:, 2:3].astype(np.int32).copy(),
        "r_off": hrt_l[:, 1:2].astype(np.int32).copy(),
        "c2": c2r,
        "adjts": adjts,
        "adjt0": adjt0,
        "a0": a0,
    }


def kernel(hrt, neb, nebr, adj, ent_W, rel_W, att_W, att_b, gcn_W, gcn_b,
           bn_gamma, bn_beta):
    from concourse.bass_utils import run_bass_kernel_spmd

    if "nc" not in _cache:
        _cache["nc"] = _build_program()
    nc = _cache["nc"]

    ent_np = np.ascontiguousarray(np.asarray(ent_W, np.float32))
    rel_np = np.ascontiguousarray(np.asarray(rel_W, np.float32))
    relw16 = np.zeros((1024, D), ml_dtypes.bfloat16)
    relw16[:NUM_RELS] = rel_np.astype(ml_dtypes.bfloat16)
    shared = {
        "ent": ent_np,
        "rel": rel_np,
        "relw16": np.ascontiguousarray(relw16.reshape(8, 128, D)),
        "att_wt": np.ascontiguousarray(np.asarray(att_W, np.float32).T),
        "att_w": np.ascontiguousarray(np.asarray(att_W, np.float32)),
        "gcn_wt": np.ascontiguousarray(np.asarray(gcn_W, np.float32).T),
        "gcn_wth": np.ascontiguousarray(np.asarray(gcn_W, np.float32).T * 0.5),
        "att_b": np.asarray(att_b, np.float32).reshape(D, 1).copy(),
        "gcn_b": np.asarray(gcn_b, np.float32).reshape(D, 1).copy(),
        "gam": np.asarray(bn_gamma, np.float32).reshape(51, 1).copy(),
        "bet": np.asarray(bn_beta, np.float32).reshape(51, 1).copy(),
    }
    in_maps = []
    for ci in range(NC):
        m = _prep_core(ci, hrt, neb, nebr, adj)
        m.update(shared)
        in_maps.append(m)
    _cache["in_maps"] = in_maps

    res = run_bass_kernel_spmd(nc, in_maps, core_ids=list(range(NC)))
    outs = res.results

    h_em = np.concatenate([o["h_out"] for o in outs])[:, None, :]
    t_em = np.concatenate([o["t_out"] for o in outs])[:, None, :]
    r_em = np.concatenate([o["r_out"] for o in outs])[:, None, :]
    y2 = np.concatenate([o["y2_out"] for o in outs])
    s2 = np.sum([o["s2_out"] for o in outs], axis=0)[0]
    m0 = s2[0] / (B * D)
    v0 = s2[1] / (B * D) - m0 * m0
    g0 = np.float32(np.asarray(bn_gamma).reshape(-1)[0])
    be0 = np.float32(np.asarray(bn_beta).reshape(-1)[0])
    tri = ((y2 - m0) / np.sqrt(v0 + EPS) * g0 + be0).astype(np.float32)[:, None, :]
    return (h_em, t_em, r_em, tri)


# revision 16
# speedup vs baseline: 1.4011x; 1.4011x over previous
"""Trainium2 Bass kernel for nn_EntityEncoder (GNN message passing encoder).

Data-parallel over batch B=1024 across 8 NeuronCores (128 batches each).
Device strategy:
  - ent_W row gathers (6400/core from the 500K x 128 table) + h/t/r lookups:
    per-partition indirect DMA gathers (order-preserving, exact f32).
  - rel_W K-mean gathers (51200 rows/core from the 1000-row table):
    TensorEngine matmul against a host-built count matrix (fp8 counts/16,
    exact) -- avoids the slow per-row Q7 descriptor path.
  - GCN bmm: per-batch PE matmuls with host-pre-transposed adjacency;
    attention softmax weights folded into the adjacency columns.
  - BatchNorm1 stats: on-device AllReduce of per-core partials. BatchNorm2
    affects only channel 0 of the output; its two global scalars are applied
    on the host.
"""
import sys

sys.path.insert(0, "/opt/trn_rl_repo")

import numpy as np
import ml_dtypes

B, N, K, D = 1024, 50, 8, 128
NUM_ENTS, NUM_RELS = 500000, 1000
NC = 8
BL = B // NC          # 128 batches per core
T = BL * N            # 6400 subg tokens per core
EPS = 1e-5
NB_CALLS = T // 128   # 50 indirect calls for neb
CH = 640              # token chunk for rel/Z matmuls (5 tiles of 128)
NCHUNK = T // CH      # 10
SC = 800              # score chunk (16 batches of 50 tokens)
NSC = T // SC         # 8

_cache = {}


def _build_program():
    import concourse.bass as bass
    import concourse.tile as tile
    from concourse import mybir, bacc

    f32 = mybir.dt.float32
    bf16 = mybir.dt.bfloat16
    fp8 = mybir.dt.float8e4
    i32 = mybir.dt.int32
    Alu = mybir.AluOpType
    Act = mybir.ActivationFunctionType

    nc = bacc.Bacc(None, target_bir_lowering=False, num_devices=NC)

    # ---- DRAM I/O ----
    ent = nc.dram_tensor("ent", [NUM_ENTS, D], f32, kind="ExternalInput")
    rel = nc.dram_tensor("rel", [NUM_RELS, D], f32, kind="ExternalInput")
    neb_off = nc.dram_tensor("neb_off", [128, NB_CALLS], i32, kind="ExternalInput")
    h_off = nc.dram_tensor("h_off", [128, 1], i32, kind="ExternalInput")
    t_off = nc.dram_tensor("t_off", [128, 1], i32, kind="ExternalInput")
    r_off = nc.dram_tensor("r_off", [128, 1], i32, kind="ExternalInput")
    c2 = nc.dram_tensor("c2", [NCHUNK, 128, 8 * CH], fp8, kind="ExternalInput")
    relw16 = nc.dram_tensor("relw16", [8, 128, D], bf16, kind="ExternalInput")
    adjts_d = nc.dram_tensor("adjts", [115, (BL // 2) * 115], f32, kind="ExternalInput")
    a0 = nc.dram_tensor("a0", [128, BL], f32, kind="ExternalInput")
    att_wt = nc.dram_tensor("att_wt", [D, D], f32, kind="ExternalInput")
    att_w = nc.dram_tensor("att_w", [D, D], f32, kind="ExternalInput")
    gcn_wt = nc.dram_tensor("gcn_wt", [D, D], f32, kind="ExternalInput")
    gcn_wth = nc.dram_tensor("gcn_wth", [D, D], f32, kind="ExternalInput")
    att_b = nc.dram_tensor("att_b", [D, 1], f32, kind="ExternalInput")
    gcn_b = nc.dram_tensor("gcn_b", [D, 1], f32, kind="ExternalInput")
    gam = nc.dram_tensor("gam", [51, 1], f32, kind="ExternalInput")
    bet = nc.dram_tensor("bet", [51, 1], f32, kind="ExternalInput")

    h_out = nc.dram_tensor("h_out", [128, D], f32, kind="ExternalOutput")
    t_out = nc.dram_tensor("t_out", [128, D], f32, kind="ExternalOutput")
    r_out = nc.dram_tensor("r_out", [128, D], f32, kind="ExternalOutput")
    y2_out = nc.dram_tensor("y2_out", [128, D], f32, kind="ExternalOutput")
    s2_out = nc.dram_tensor("s2_out", [1, 2], f32, kind="ExternalOutput")

    with tile.TileContext(nc) as tc:
        with (
            tc.tile_pool(name="big", bufs=1) as big,
            tc.tile_pool(name="med", bufs=1) as med,
            tc.tile_pool(name="ring", bufs=2) as ring,
            tc.tile_pool(name="tiny", bufs=1) as tiny,
            tc.tile_pool(name="psA", bufs=2, space="PSUM") as psA,
            tc.tile_pool(name="psB", bufs=2, space="PSUM") as psB,
            tc.tile_pool(name="psC", bufs=2, space="PSUM") as psC,
            tc.tile_pool(name="dram", bufs=1, space="DRAM") as dpool,
        ):
            # ---- constants ----
            def scaled_ident(val, nm):
                t_ = tiny.tile([128, 128], f32, tag=nm)
                nc.gpsimd.memset(t_[:], 0.0)
                nc.gpsimd.affine_select(
                    out=t_[:], in_=t_[:], compare_op=Alu.not_equal, fill=val,
                    base=0, pattern=[[-1, 128]], channel_multiplier=1)
                return t_

            ident = scaled_ident(1.0, "id1")
            onescol = tiny.tile([128, 1], f32, tag="ones")
            nc.vector.memset(onescol[:], 1.0)

            def preload(dram_t, shape, nm, dt_=f32):
                t_ = tiny.tile(shape, dt_, tag=nm)
                nc.sync.dma_start(out=t_[:], in_=dram_t[:])
                return t_

            attwt_sb = preload(att_wt, [D, D], "attwt")
            attw_sb = preload(att_w, [D, D], "attw")
            gcnwt_sb = preload(gcn_wt, [D, D], "gcnwt")
            gcnwth_sb = preload(gcn_wth, [D, D], "gcnwth")
            attb_sb = preload(att_b, [D, 1], "attb")
            gcnb_sb = preload(gcn_b, [D, 1], "gcnb")
            gam_sb = preload(gam, [51, 1], "gam")
            bet_sb = preload(bet, [51, 1], "bet")
            a0_sb = preload(a0, [128, BL], "a0")
            nebo_sb = preload(neb_off, [128, NB_CALLS], "nebo", i32)
            ho_sb = preload(h_off, [128, 1], "ho", i32)
            to_sb = preload(t_off, [128, 1], "to", i32)
            ro_sb = preload(r_off, [128, 1], "ro", i32)
            relw_sb = tiny.tile([128, 8, D], bf16, tag="relw")
            for s in range(8):
                nc.sync.dma_start(out=relw_sb[:, s, :], in_=relw16[s, :, :])
            adjts_sb = med.tile([115, (BL // 2) * 115], f32, tag="adjts")
            nc.sync.dma_start(out=adjts_sb[:], in_=adjts_d[:])

            # ---- h/t/r gathers (token-major, exact f32) ----
            def igather(dst, table, offs):
                nc.gpsimd.indirect_dma_start(
                    out=dst, out_offset=None, in_=table[:],
                    in_offset=bass.IndirectOffsetOnAxis(ap=offs, axis=0))

            hg = med.tile([128, D], f32, tag="hg")
            tg = med.tile([128, D], f32, tag="tg")
            rg = med.tile([128, D], f32, tag="rg")
            igather(hg[:], ent, ho_sb[:, 0:1])
            igather(tg[:], ent, to_sb[:, 0:1])
            igather(rg[:], rel, ro_sb[:, 0:1])
            nc.sync.dma_start(out=h_out[:], in_=hg[:])
            nc.sync.dma_start(out=t_out[:], in_=tg[:])
            nc.sync.dma_start(out=r_out[:], in_=rg[:])

            # S = (h+t+r)/3 D-major via 3 accumulating scaled transposes
            s_ps = psC.tile([128, 128], f32, tag="sc", space="PSUM")
            nc.tensor.matmul(out=s_ps[:], lhsT=hg[:], rhs=ident[:],
                             is_transpose=True, start=True, stop=False)
            nc.tensor.matmul(out=s_ps[:], lhsT=tg[:], rhs=ident[:],
                             is_transpose=True, start=False, stop=False)
            nc.tensor.matmul(out=s_ps[:], lhsT=rg[:], rhs=ident[:],
                             is_transpose=True, start=False, stop=True)
            s_sb = med.tile([128, 128], f32, tag="ssb")
            nc.vector.tensor_scalar_mul(out=s_sb[:], in0=s_ps[:],
                                        scalar1=1.0 / 3.0)

            # ---- neb gather: 50 indirect calls, token-major ----
            nebg = big.tile([128, NB_CALLS, D], f32, tag="bb")
            for j in range(NB_CALLS):
                igather(nebg[:, j, :], ent, nebo_sb[:, j:j + 1])

            # ---- subgT = 0.5*neb_eT + (1/16)*sum_k rel_W[nebr] ----
            subgT = big.tile([128, T], f32, tag="subgT")
            for ch in range(NCHUNK):
                c2f8 = ring.tile([128, 8 * CH], fp8, tag="c2f8")
                nc.sync.dma_start(out=c2f8[:], in_=c2[ch, :, :])
                rel_ps = psA.tile([128, CH], f32, tag="mm", space="PSUM")
                for s in range(8):
                    c2bf = ring.tile([128, CH], bf16, tag="c2bf")
                    nc.vector.tensor_copy(
                        out=c2bf[:], in_=c2f8[:, s * CH:(s + 1) * CH])
                    nc.tensor.matmul(out=rel_ps[:, 0:512],
                                     lhsT=relw_sb[:, s, :], rhs=c2bf[:, 0:512],
                                     start=(s == 0), stop=(s == 7))
                    nc.tensor.matmul(out=rel_ps[:, 512:CH],
                                     lhsT=relw_sb[:, s, :], rhs=c2bf[:, 512:CH],
                                     start=(s == 0), stop=(s == 7))
                relc_sb = ring.tile([128, CH], f32, tag="relc")
                nc.vector.tensor_copy(out=relc_sb[:], in_=rel_ps[:])
                for jj in range(CH // 128):
                    j = ch * (CH // 128) + jj
                    tr_ps = psB.tile([128, 128], f32, tag="tr", space="PSUM")
                    nc.tensor.matmul(out=tr_ps[:], lhsT=nebg[:, j, :],
                                     rhs=ident[:], is_transpose=True,
                                     start=True, stop=True)
                    nc.vector.tensor_tensor(
                        out=subgT[:, j * 128:(j + 1) * 128],
                        in0=tr_ps[:], in1=relc_sb[:, jj * 128:(jj + 1) * 128],
                        op=Alu.add)

            # ---- Q, U, c ----
            q_ps = psA.tile([128, 128], f32, tag="mm", space="PSUM")
            nc.tensor.matmul(out=q_ps[:], lhsT=attwt_sb[:], rhs=s_sb[:],
                             start=True, stop=True)
            q_sb = med.tile([128, 128], f32, tag="qsb")
            nc.scalar.activation(out=q_sb[:], in_=q_ps[:], func=Act.Identity,
                                 bias=attb_sb[:, 0:1], scale=1.0)
            u_ps = psA.tile([128, 128], f32, tag="mm", space="PSUM")
            nc.tensor.matmul(out=u_ps[:], lhsT=attw_sb[:], rhs=q_sb[:],
                             start=True, stop=True)
            u_sb = med.tile([128, 128], f32, tag="usb")
            nc.vector.tensor_scalar_mul(out=u_sb[:], in0=u_ps[:], scalar1=0.5)
            c_ps = psC.tile([1, 128], f32, tag="sc", space="PSUM")
            nc.tensor.matmul(out=c_ps[:], lhsT=attb_sb[:, 0:1], rhs=q_sb[:],
                             start=True, stop=True)
            c_sb = tiny.tile([1, 128], f32, tag="csb")
            nc.vector.tensor_copy(out=c_sb[:], in_=c_ps[:])
            ct_sb = tiny.tile([128, 1], f32, tag="ctsb")
            nc.sync.dma_start(out=ct_sb[:], in_=c_sb[:])

            # ---- attention scores ----
            scores_dr = dpool.tile([1, T], f32)
            for ch in range(NSC):           # 800 tokens = 16 batches
                prod = ring.tile([128, SC], f32, tag="prod")
                nc.vector.tensor_tensor(
                    out=prod[:].rearrange("p (b n) -> p b n", n=N),
                    in0=subgT[:, ch * SC:(ch + 1) * SC].rearrange(
                        "p (b n) -> p b n", n=N),
                    in1=u_sb[:, ch * 16:(ch + 1) * 16].rearrange(
                        "p (b one) -> p b one", one=1).to_broadcast([128, 16, N]),
                    op=Alu.mult)
                sc_sb = med.tile([1, SC], f32, tag="scsb")
                for (o0, o1) in ((0, 512), (512, SC)):
                    sc_ps = psC.tile([1, 512], f32, tag="sc", space="PSUM")
                    nc.tensor.matmul(out=sc_ps[:, :o1 - o0], lhsT=onescol[:],
                                     rhs=prod[:, o0:o1], start=True, stop=True)
                    nc.vector.tensor_copy(out=sc_sb[:, o0:o1],
                                          in_=sc_ps[:, :o1 - o0])
                nc.sync.dma_start(out=scores_dr[:, ch * SC:(ch + 1) * SC],
                                  in_=sc_sb[:])
            scoresb = med.tile([128, N], f32, tag="scoresb")
            nc.sync.dma_start(out=scoresb[:], in_=scores_dr[:])
            nc.vector.tensor_scalar_add(out=scoresb[:], in0=scoresb[:],
                                        scalar1=ct_sb[:, 0:1])
            nc.scalar.activation(out=scoresb[:], in_=scoresb[:], func=Act.Lrelu,
                                 alpha=0.01)
            esum = tiny.tile([128, 1], f32, tag="esum")
            nc.scalar.activation(out=scoresb[:], in_=scoresb[:], func=Act.Exp,
                                 accum_out=esum[:, 0:1])
            einv = tiny.tile([128, 1], f32, tag="einv")
            nc.vector.reciprocal(out=einv[:], in_=esum[:])
            nc.vector.tensor_scalar_mul(out=scoresb[:], in0=scoresb[:],
                                        scalar1=einv[:, 0:1])

            # att weights transposed [50, 128]; fold into adjts columns
            att_ps = psB.tile([N, 128], f32, tag="tr", space="PSUM")
            nc.tensor.matmul(out=att_ps[:], lhsT=scoresb[:], rhs=ident[:],
                             is_transpose=True, start=True, stop=True)
            atts = med.tile([N, 128], f32, tag="atts")
            nc.vector.tensor_copy(out=atts[:], in_=att_ps[:])
            # pair scales [115, 64]: rows 0..49 = att(even b), 64..113 =
            # att(odd b); rows 50/114 (sum_hrt tokens) stay 1.0
            attsp = med.tile([115, BL // 2], f32, tag="attsp")
            nc.vector.memset(attsp[:], 1.0)
            attev = atts[:].rearrange("p (j two) -> p j two", two=2)
            nc.vector.tensor_copy(
                out=attsp[0:50, :].rearrange("p (j one) -> p j one", one=1),
                in_=attev[:, :, 0:1])
            nc.vector.tensor_copy(
                out=attsp[64:114, :].rearrange("p (j one) -> p j one", one=1),
                in_=attev[:, :, 1:2])
            nc.vector.tensor_tensor(
                out=adjts_sb[:].rearrange("p (j i) -> p j i", j=BL // 2),
                in0=adjts_sb[:].rearrange("p (j i) -> p j i", j=BL // 2),
                in1=attsp[:].rearrange("p (j one) -> p j one", one=1
                                       ).to_broadcast([115, BL // 2, 115]),
                op=Alu.mult)

            # ---- Z = gcn_W @ [S | subgT] + gcn_b, laid out 51 tokens/b ----
            z51 = big.tile([128, BL * 51], f32, tag="bb")  # reuses nebg slot
            z51v = z51[:].rearrange("p (b c) -> p b c", c=51)
            zs_ps = psA.tile([128, 128], f32, tag="mm", space="PSUM")
            nc.tensor.matmul(out=zs_ps[:], lhsT=gcnwt_sb[:], rhs=s_sb[:],
                             start=True, stop=True)
            nc.scalar.activation(
                out=z51v[:, :, 50:51],
                in_=zs_ps[:].rearrange("p (b one) -> p b one", one=1),
                func=Act.Identity, bias=gcnb_sb[:, 0:1], scale=1.0)
            for ch in range(NSC):          # 800-token chunks = 16 batches
                z_ps = psA.tile([128, SC], f32, tag="mm", space="PSUM")
                nc.tensor.matmul(out=z_ps[:, 0:512], lhsT=gcnwth_sb[:],
                                 rhs=subgT[:, ch * SC:ch * SC + 512],
                                 start=True, stop=True)
                nc.tensor.matmul(out=z_ps[:, 512:SC], lhsT=gcnwth_sb[:],
                                 rhs=subgT[:, ch * SC + 512:(ch + 1) * SC],
                                 start=True, stop=True)
                nc.scalar.activation(
                    out=z51v[:, ch * 16:(ch + 1) * 16, 0:50],
                    in_=z_ps[:].rearrange("p (b n) -> p b n", n=N),
                    func=Act.Identity, bias=gcnb_sb[:, 0:1], scale=1.0)

            # ---- bmm1 + relu + stats (pairs at partitions 0 / 64) ----
            y1 = big.tile([128, (BL // 2) * 128], f32, tag="y1")
            sum1 = tiny.tile([128, 16], f32, tag="sum1")
            sum2 = tiny.tile([128, 16], f32, tag="sum2")
            for g8 in range(BL // 8):      # 4 pairs per iteration
                ztq_ev = psB.tile([51, 512], f32, tag="tr", space="PSUM")
                ztq_od = psB.tile([51, 512], f32, tag="tr", space="PSUM")
                for jj in range(4):
                    j = g8 * 4 + jj
                    nc.tensor.matmul(
                        out=ztq_ev[:, jj * 128:(jj + 1) * 128],
                        lhsT=z51[:, (2 * j) * 51:(2 * j + 1) * 51],
                        rhs=ident[:], is_transpose=True, start=True, stop=True)
                    nc.tensor.matmul(
                        out=ztq_od[:, jj * 128:(jj + 1) * 128],
                        lhsT=z51[:, (2 * j + 1) * 51:(2 * j + 2) * 51],
                        rhs=ident[:], is_transpose=True, start=True, stop=True)
                zt_sb = med.tile([128, 512], f32, tag="zt")
                nc.vector.memset(zt_sb[:], 0.0)
                nc.vector.tensor_copy(out=zt_sb[0:51, :], in_=ztq_ev[:])
                nc.vector.tensor_copy(out=zt_sb[64:115, :], in_=ztq_od[:])
                y1_ps = psA.tile([128, 512], f32, tag="mm", space="PSUM")
                for jj in range(4):
                    j = g8 * 4 + jj
                    nc.tensor.matmul(
                        out=y1_ps[0:115, jj * 128:(jj + 1) * 128],
                        lhsT=adjts_sb[:, j * 115:(j + 1) * 115],
                        rhs=zt_sb[0:115, jj * 128:(jj + 1) * 128],
                        start=True, stop=True)
                nc.scalar.activation(out=y1[:, g8 * 512:(g8 + 1) * 512],
                                     in_=y1_ps[:], func=Act.Relu,
                                     accum_out=sum1[:, g8:g8 + 1])
                sqbuf = med.tile([128, 512], f32, tag="sq")
                nc.scalar.activation(out=sqbuf[:],
                                     in_=y1[:, g8 * 512:(g8 + 1) * 512],
                                     func=Act.Square,
                                     accum_out=sum2[:, g8:g8 + 1])

            # ---- BN1 stats fold + AllReduce ----
            Ax = mybir.AxisListType
            st = tiny.tile([128, 2], f32, tag="st")
            nc.vector.tensor_reduce(out=st[:, 0:1], in_=sum1[:], axis=Ax.X,
                                    op=Alu.add)
            nc.vector.tensor_reduce(out=st[:, 1:2], in_=sum2[:], axis=Ax.X,
                                    op=Alu.add)
            sthi = tiny.tile([51, 2], f32, tag="sthi")
            nc.vector.tensor_copy(out=sthi[:], in_=st[64:115, :])
            stf = tiny.tile([51, 2], f32, tag="stf")
            nc.vector.tensor_tensor(out=stf[:], in0=st[0:51, :],
                                    in1=sthi[:], op=Alu.add)
            bnc_in = dpool.tile([51, 2], f32)
            bnc_out = dpool.tile([51, 2], f32)
            nc.gpsimd.dma_start(out=bnc_in[:], in_=stf[:])
            nc.gpsimd.collective_compute(
                "AllReduce", Alu.add, replica_groups=[list(range(NC))],
                ins=[bnc_in.opt()], outs=[bnc_out.opt()])
            stg = tiny.tile([51, 2], f32, tag="stg")
            nc.gpsimd.dma_start(out=stg[:], in_=bnc_out[:])

            inv_n = 1.0 / (B * 128.0)
            mcol = tiny.tile([51, 1], f32, tag="mcol")
            nc.vector.tensor_scalar_mul(out=mcol[:], in0=stg[:, 0:1],
                                        scalar1=inv_n)
            m2 = tiny.tile([51, 1], f32, tag="m2")
            nc.vector.tensor_mul(m2[:], mcol[:], mcol[:])
            vcol = tiny.tile([51, 1], f32, tag="vcol")
            nc.vector.tensor_scalar_mul(out=vcol[:], in0=stg[:, 1:2],
                                        scalar1=inv_n)
            nc.vector.tensor_sub(vcol[:], vcol[:], m2[:])
            epst = tiny.tile([51, 1], f32, tag="epst")
            nc.vector.memset(epst[:], EPS)
            sd = tiny.tile([51, 1], f32, tag="sd")
            nc.scalar.activation(out=sd[:], in_=vcol[:], func=Act.Sqrt,
                                 bias=epst[:, 0:1])
            rstd = tiny.tile([51, 1], f32, tag="rstd")
            nc.vector.reciprocal(out=rstd[:], in_=sd[:])
            alpha = tiny.tile([51, 1], f32, tag="alpha")
            nc.vector.tensor_mul(alpha[:], gam_sb[:], rstd[:])
            ccol = tiny.tile([51, 1], f32, tag="ccol")
            nc.vector.tensor_mul(ccol[:], mcol[:], alpha[:])
            nc.vector.tensor_sub(ccol[:], bet_sb[:], ccol[:])
            alpha2 = tiny.tile([128, 1], f32, tag="alpha2")
            nc.vector.memset(alpha2[:], 0.0)
            nc.vector.tensor_copy(out=alpha2[0:51, :], in_=alpha[:])
            nc.vector.tensor_copy(out=alpha2[64:115, :], in_=alpha[:])
            ccol2 = tiny.tile([128, 1], f32, tag="ccol2")
            nc.vector.memset(ccol2[:], 0.0)
            nc.vector.tensor_copy(out=ccol2[0:51, :], in_=ccol[:])
            nc.vector.tensor_copy(out=ccol2[64:115, :], in_=ccol[:])

            nc.vector.tensor_scalar(y1[:], y1[:], alpha2[:, 0:1],
                                    ccol2[:, 0:1], Alu.mult, Alu.add)

            # ---- bmm2 row 0 (column-major out) + relu + stats2 ----
            y2t_ps = psB.tile([128, 128], f32, tag="tr", space="PSUM")
            for j in range(BL // 2):
                nc.tensor.matmul(
                    out=y2t_ps[:, 2 * j:2 * j + 2],
                    lhsT=y1[0:115, j * 128:(j + 1) * 128],
                    rhs=a0_sb[0:115, 2 * j:2 * j + 2],
                    start=True, stop=True)
            p1 = tiny.tile([128, 1], f32, tag="p1")
            y2t_sb = med.tile([128, 128], f32, tag="y2t")
            nc.scalar.activation(out=y2t_sb[:], in_=y2t_ps[:], func=Act.Relu,
                                 accum_out=p1[:, 0:1])
            p2 = tiny.tile([128, 1], f32, tag="p2")
            y2sq = med.tile([128, 128], f32, tag="y2sq")
            nc.scalar.activation(out=y2sq[:], in_=y2t_sb[:], func=Act.Square,
                                 accum_out=p2[:, 0:1])
            # transpose to token-major for output
            y2m_ps = psC.tile([128, 128], f32, tag="sc", space="PSUM")
            nc.tensor.matmul(out=y2m_ps[:], lhsT=y2t_sb[:], rhs=ident[:],
                             is_transpose=True, start=True, stop=True)
            y2m_sb = med.tile([128, 128], f32, tag="y2m")
            nc.vector.tensor_copy(out=y2m_sb[:], in_=y2m_ps[:])
            nc.sync.dma_start(out=y2_out[:], in_=y2m_sb[:])

            pc = tiny.tile([128, 2], f32, tag="pc")
            nc.vector.tensor_copy(out=pc[:, 0:1], in_=p1[:])
            nc.vector.tensor_copy(out=pc[:, 1:2], in_=p2[:])
            s2_ps = psC.tile([1, 2], f32, tag="sc", space="PSUM")
            nc.tensor.matmul(out=s2_ps[:], lhsT=onescol[:], rhs=pc[:],
                             start=True, stop=True)
            s2_sb = tiny.tile([1, 2], f32, tag="s2sb")
            nc.vector.tensor_copy(out=s2_sb[:], in_=s2_ps[:])
            nc.sync.dma_start(out=s2_out[:], in_=s2_sb[:])

    nc.finalize()
    return nc


def _prep_core(ci, hrt, neb, nebr, adj):
    b0, b1 = ci * BL, (ci + 1) * BL
    hrt_l = np.asarray(hrt[b0:b1], np.int64)
    neb_l = np.asarray(neb[b0:b1], np.int64)
    nebr_l = np.asarray(nebr[b0:b1], np.int64)
    adj_l = np.asarray(adj[b0:b1], np.float32)

    neb_flat = neb_l.reshape(T)
    neb_off = np.ascontiguousarray(neb_flat.reshape(NB_CALLS, 128).T).astype(np.int32)

    tok = np.repeat(np.arange(T, dtype=np.int64), K)
    rr = nebr_l.reshape(T * K)
    cnt = np.zeros((1024, T), np.float32)
    np.add.at(cnt, (rr, tok), 1.0)
    cnt *= (1.0 / 8.0)
    c2r = cnt.reshape(8, 128, NCHUNK, CH).transpose(2, 1, 0, 3).reshape(
        NCHUNK, 128, 8 * CH)
    c2r = np.ascontiguousarray(c2r).astype(ml_dtypes.float8_e4m3)

    # pair-blocked lhsT: [102, 64, 115]; K rows 0..50 = batch 2j channels,
    # 51..101 = batch 2j+1; M cols 0..50 / 64..114 = the two output bands
    AT = adj_l.transpose(0, 2, 1)                   # AT[b, k, i] = adj[b, i, k]
    ATr = np.concatenate([AT[:, 1:51, :], AT[:, 0:1, :]], axis=1)  # token order
    ap_ = np.zeros((115, BL // 2, 115), np.float32)
    ap_[0:51, :, 0:51] = ATr[0::2].transpose(1, 0, 2)
    ap_[64:115, :, 64:115] = ATr[1::2].transpose(1, 0, 2)
    adjts = np.ascontiguousarray(ap_.reshape(115, (BL // 2) * 115))
    a0 = np.zeros((128, BL), np.float32)
    a0[0:51, 0::2] = adj_l[0::2, 0, :].T
    a0[64:115, 1::2] = adj_l[1::2, 0, :].T

    return {
        "neb_off": neb_off,
        "h_off": hrt_l[:, 0:1].astype(np.int32).copy(),
        "t_off": hrt_l[:, 2:3].astype(np.int32).copy(),
        "r_off": hrt_l[:, 1:2].astype(np.int32).copy(),
        "c2": c2r,
        "adjts": adjts,
        "a0": a0,
    }


def kernel(hrt, neb, nebr, adj, ent_W, rel_W, att_W, att_b, gcn_W, gcn_b,
           bn_gamma, bn_beta):
    from concourse.bass_utils import run_bass_kernel_spmd

    if "nc" not in _cache:
        _cache["nc"] = _build_program()
    nc = _cache["nc"]

    ent_np = np.ascontiguousarray(np.asarray(ent_W, np.float32))
    rel_np = np.ascontiguousarray(np.asarray(rel_W, np.float32))
    relw16 = np.zeros((1024, D), ml_dtypes.bfloat16)
    relw16[:NUM_RELS] = rel_np.astype(ml_dtypes.bfloat16)
    shared = {
        "ent": ent_np,
        "rel": rel_np,
        "relw16": np.ascontiguousarray(relw16.reshape(8, 128, D)),
        "att_wt": np.ascontiguousarray(np.asarray(att_W, np.float32).T),
        "att_w": np.ascontiguousarray(np.asarray(att_W, np.float32)),
        "gcn_wt": np.ascontiguousarray(np.asarray(gcn_W, np.float32).T),
        "gcn_wth": np.ascontiguousarray(np.asarray(gcn_W, np.float32).T * 0.5),
        "att_b": np.asarray(att_b, np.float32).reshape(D, 1).copy(),
        "gcn_b": np.asarray(gcn_b, np.float32).reshape(D, 1).copy(),
        "gam": np.asarray(bn_gamma, np.float32).reshape(51, 1).copy(),
        "bet": np.asarray(bn_beta, np.float32).reshape(51, 1).copy(),
    }
    in_maps = []
    for ci in range(NC):
        m = _prep_core(ci, hrt, neb, nebr, adj)
        m.update(shared)
        in_maps.append(m)
    _cache["in_maps"] = in_maps

    res = run_bass_kernel_spmd(nc, in_maps, core_ids=list(range(NC)))
    outs = res.results

    h_em = np.concatenate([o["h_out"] for o in outs])[:, None, :]
    t_em = np.concatenate([o["t_out"] for o in outs])[:, None, :]
    r_em = np.concatenate([o["r_out"] for o in outs])[:, None, :]
    y2 = np.concatenate([o["y2_out"] for o in outs])
    s2 = np.sum([o["s2_out"] for o in outs], axis=0)[0]
    m0 = s2[0] / (B * D)
    v0 = s2[1] / (B * D) - m0 * m0
    g0 = np.float32(np.asarray(bn_gamma).reshape(-1)[0])
    be0 = np.float32(np.asarray(bn_beta).reshape(-1)[0])
    tri = ((y2 - m0) / np.sqrt(v0 + EPS) * g0 + be0).astype(np.float32)[:, None, :]
    return (h_em, t_em, r_em, tri)


# revision 17
# speedup vs baseline: 1.4337x; 1.0233x over previous
"""Trainium2 Bass kernel for nn_EntityEncoder (GNN message passing encoder).

Data-parallel over batch B=1024 across 8 NeuronCores (128 batches each).
Device strategy:
  - ent_W row gathers (6400/core from the 500K x 128 table) + h/t/r lookups:
    per-partition indirect DMA gathers (order-preserving, exact f32).
  - rel_W K-mean gathers (51200 rows/core from the 1000-row table):
    TensorEngine matmul against a host-built count matrix (fp8 counts/16,
    exact) -- avoids the slow per-row Q7 descriptor path.
  - GCN bmm: per-batch PE matmuls with host-pre-transposed adjacency;
    attention softmax weights folded into the adjacency columns.
  - BatchNorm1 stats: on-device AllReduce of per-core partials. BatchNorm2
    affects only channel 0 of the output; its two global scalars are applied
    on the host.
"""
import sys

sys.path.insert(0, "/opt/trn_rl_repo")

import numpy as np
import ml_dtypes

B, N, K, D = 1024, 50, 8, 128
NUM_ENTS, NUM_RELS = 500000, 1000
NC = 8
BL = B // NC          # 128 batches per core
T = BL * N            # 6400 subg tokens per core
EPS = 1e-5
NB_CALLS = T // 128   # 50 indirect calls for neb
CH = 640              # token chunk for rel/Z matmuls (5 tiles of 128)
NCHUNK = T // CH      # 10
SC = 800              # score chunk (16 batches of 50 tokens)
NSC = T // SC         # 8

_cache = {}


def _build_program():
    import concourse.bass as bass
    import concourse.tile as tile
    from concourse import mybir, bacc

    f32 = mybir.dt.float32
    bf16 = mybir.dt.bfloat16
    fp8 = mybir.dt.float8e4
    i32 = mybir.dt.int32
    Alu = mybir.AluOpType
    Act = mybir.ActivationFunctionType

    nc = bacc.Bacc(None, target_bir_lowering=False, num_devices=NC)

    # ---- DRAM I/O ----
    ent = nc.dram_tensor("ent", [NUM_ENTS, D], f32, kind="ExternalInput")
    rel = nc.dram_tensor("rel", [NUM_RELS, D], f32, kind="ExternalInput")
    neb_off = nc.dram_tensor("neb_off", [128, NB_CALLS], i32, kind="ExternalInput")
    h_off = nc.dram_tensor("h_off", [128, 1], i32, kind="ExternalInput")
    t_off = nc.dram_tensor("t_off", [128, 1], i32, kind="ExternalInput")
    r_off = nc.dram_tensor("r_off", [128, 1], i32, kind="ExternalInput")
    c2 = nc.dram_tensor("c2", [NCHUNK, 128, 8 * CH], fp8, kind="ExternalInput")
    relw16 = nc.dram_tensor("relw16", [8, 128, D], bf16, kind="ExternalInput")
    adjts_d = nc.dram_tensor("adjts", [115, (BL // 2) * 115], f32, kind="ExternalInput")
    a0 = nc.dram_tensor("a0", [128, BL], f32, kind="ExternalInput")
    att_wt = nc.dram_tensor("att_wt", [D, D], f32, kind="ExternalInput")
    att_w = nc.dram_tensor("att_w", [D, D], f32, kind="ExternalInput")
    gcn_wt = nc.dram_tensor("gcn_wt", [D, D], f32, kind="ExternalInput")
    gcn_wth = nc.dram_tensor("gcn_wth", [D, D], f32, kind="ExternalInput")
    att_b = nc.dram_tensor("att_b", [D, 1], f32, kind="ExternalInput")
    gcn_b = nc.dram_tensor("gcn_b", [D, 1], f32, kind="ExternalInput")
    gam = nc.dram_tensor("gam", [51, 1], f32, kind="ExternalInput")
    bet = nc.dram_tensor("bet", [51, 1], f32, kind="ExternalInput")

    h_out = nc.dram_tensor("h_out", [128, D], f32, kind="ExternalOutput")
    t_out = nc.dram_tensor("t_out", [128, D], f32, kind="ExternalOutput")
    r_out = nc.dram_tensor("r_out", [128, D], f32, kind="ExternalOutput")
    y2_out = nc.dram_tensor("y2_out", [128, D], f32, kind="ExternalOutput")
    s2_out = nc.dram_tensor("s2_out", [1, 2], f32, kind="ExternalOutput")

    with tile.TileContext(nc) as tc:
        with (
            tc.tile_pool(name="big", bufs=1) as big,
            tc.tile_pool(name="med", bufs=1) as med,
            tc.tile_pool(name="ring", bufs=2) as ring,
            tc.tile_pool(name="tiny", bufs=1) as tiny,
            tc.tile_pool(name="psA", bufs=2, space="PSUM") as psA,
            tc.tile_pool(name="psB", bufs=2, space="PSUM") as psB,
            tc.tile_pool(name="psC", bufs=2, space="PSUM") as psC,
            tc.tile_pool(name="dram", bufs=1, space="DRAM") as dpool,
        ):
            # ---- constants ----
            def scaled_ident(val, nm):
                t_ = tiny.tile([128, 128], f32, tag=nm)
                nc.gpsimd.memset(t_[:], 0.0)
                nc.gpsimd.affine_select(
                    out=t_[:], in_=t_[:], compare_op=Alu.not_equal, fill=val,
                    base=0, pattern=[[-1, 128]], channel_multiplier=1)
                return t_

            ident = scaled_ident(1.0, "id1")
            onescol = tiny.tile([128, 1], f32, tag="ones")
            nc.vector.memset(onescol[:], 1.0)

            def preload(dram_t, shape, nm, dt_=f32):
                t_ = tiny.tile(shape, dt_, tag=nm)
                nc.sync.dma_start(out=t_[:], in_=dram_t[:])
                return t_

            attwt_sb = preload(att_wt, [D, D], "attwt")
            attw_sb = preload(att_w, [D, D], "attw")
            gcnwt_sb = preload(gcn_wt, [D, D], "gcnwt")
            gcnwth_sb = preload(gcn_wth, [D, D], "gcnwth")
            attb_sb = preload(att_b, [D, 1], "attb")
            gcnb_sb = preload(gcn_b, [D, 1], "gcnb")
            gam_sb = preload(gam, [51, 1], "gam")
            bet_sb = preload(bet, [51, 1], "bet")
            a0_sb = preload(a0, [128, BL], "a0")
            nebo_sb = preload(neb_off, [128, NB_CALLS], "nebo", i32)
            ho_sb = preload(h_off, [128, 1], "ho", i32)
            to_sb = preload(t_off, [128, 1], "to", i32)
            ro_sb = preload(r_off, [128, 1], "ro", i32)
            relw_sb = tiny.tile([128, 8, D], bf16, tag="relw")
            for s in range(8):
                nc.sync.dma_start(out=relw_sb[:, s, :], in_=relw16[s, :, :])
            adjts_sb = med.tile([115, (BL // 2) * 115], f32, tag="adjts")
            nc.sync.dma_start(out=adjts_sb[:], in_=adjts_d[:])

            # ---- h/t/r gathers (token-major, exact f32) ----
            def igather(dst, table, offs):
                nc.gpsimd.indirect_dma_start(
                    out=dst, out_offset=None, in_=table[:],
                    in_offset=bass.IndirectOffsetOnAxis(ap=offs, axis=0))

            hg = med.tile([128, D], f32, tag="hg")
            tg = med.tile([128, D], f32, tag="tg")
            rg = med.tile([128, D], f32, tag="rg")
            igather(hg[:], ent, ho_sb[:, 0:1])
            igather(tg[:], ent, to_sb[:, 0:1])
            igather(rg[:], rel, ro_sb[:, 0:1])
            nc.sync.dma_start(out=h_out[:], in_=hg[:])
            nc.sync.dma_start(out=t_out[:], in_=tg[:])
            nc.sync.dma_start(out=r_out[:], in_=rg[:])

            # S = (h+t+r)/3 D-major via 3 accumulating scaled transposes
            s_ps = psC.tile([128, 128], f32, tag="sc", space="PSUM")
            nc.tensor.matmul(out=s_ps[:], lhsT=hg[:], rhs=ident[:],
                             is_transpose=True, start=True, stop=False)
            nc.tensor.matmul(out=s_ps[:], lhsT=tg[:], rhs=ident[:],
                             is_transpose=True, start=False, stop=False)
            nc.tensor.matmul(out=s_ps[:], lhsT=rg[:], rhs=ident[:],
                             is_transpose=True, start=False, stop=True)
            s_sb = med.tile([128, 128], f32, tag="ssb")
            nc.vector.tensor_scalar_mul(out=s_sb[:], in0=s_ps[:],
                                        scalar1=1.0 / 3.0)

            # ---- neb gather: 50 indirect calls, token-major ----
            nebg = big.tile([128, NB_CALLS, D], f32, tag="bb")
            for j in range(NB_CALLS):
                igather(nebg[:, j, :], ent, nebo_sb[:, j:j + 1])

            # ---- subgT = 0.5*neb_eT + (1/16)*sum_k rel_W[nebr] ----
            subgT = big.tile([128, T], f32, tag="subgT")
            for ch in range(NCHUNK):
                c2f8 = ring.tile([128, 8 * CH], fp8, tag="c2f8")
                nc.sync.dma_start(out=c2f8[:], in_=c2[ch, :, :])
                rel_ps = psA.tile([128, CH], f32, tag="mm", space="PSUM")
                for s in range(8):
                    c2bf = ring.tile([128, CH], bf16, tag="c2bf")
                    nc.vector.tensor_copy(
                        out=c2bf[:], in_=c2f8[:, s * CH:(s + 1) * CH])
                    nc.tensor.matmul(out=rel_ps[:, 0:512],
                                     lhsT=relw_sb[:, s, :], rhs=c2bf[:, 0:512],
                                     start=(s == 0), stop=(s == 7))
                    nc.tensor.matmul(out=rel_ps[:, 512:CH],
                                     lhsT=relw_sb[:, s, :], rhs=c2bf[:, 512:CH],
                                     start=(s == 0), stop=(s == 7))
                relc_sb = ring.tile([128, CH], f32, tag="relc")
                nc.vector.tensor_copy(out=relc_sb[:], in_=rel_ps[:])
                for jj in range(CH // 128):
                    j = ch * (CH // 128) + jj
                    tr_ps = psB.tile([128, 128], f32, tag="tr", space="PSUM")
                    nc.tensor.matmul(out=tr_ps[:], lhsT=nebg[:, j, :],
                                     rhs=ident[:], is_transpose=True,
                                     start=True, stop=True)
                    nc.vector.tensor_tensor(
                        out=subgT[:, j * 128:(j + 1) * 128],
                        in0=tr_ps[:], in1=relc_sb[:, jj * 128:(jj + 1) * 128],
                        op=Alu.add)

            # ---- Q, U, c ----
            q_ps = psA.tile([128, 128], f32, tag="mm", space="PSUM")
            nc.tensor.matmul(out=q_ps[:], lhsT=attwt_sb[:], rhs=s_sb[:],
                             start=True, stop=True)
            q_sb = med.tile([128, 128], f32, tag="qsb")
            nc.scalar.activation(out=q_sb[:], in_=q_ps[:], func=Act.Identity,
                                 bias=attb_sb[:, 0:1], scale=1.0)
            u_ps = psA.tile([128, 128], f32, tag="mm", space="PSUM")
            nc.tensor.matmul(out=u_ps[:], lhsT=attw_sb[:], rhs=q_sb[:],
                             start=True, stop=True)
            u_sb = med.tile([128, 128], f32, tag="usb")
            nc.vector.tensor_scalar_mul(out=u_sb[:], in0=u_ps[:], scalar1=0.5)
            c_ps = psC.tile([1, 128], f32, tag="sc", space="PSUM")
            nc.tensor.matmul(out=c_ps[:], lhsT=attb_sb[:, 0:1], rhs=q_sb[:],
                             start=True, stop=True)
            c_sb = tiny.tile([1, 128], f32, tag="csb")
            nc.vector.tensor_copy(out=c_sb[:], in_=c_ps[:])
            ct_sb = tiny.tile([128, 1], f32, tag="ctsb")
            nc.sync.dma_start(out=ct_sb[:], in_=c_sb[:])

            # ---- attention scores ----
            scores_dr = dpool.tile([1, T], f32)
            for ch in range(NSC):           # 800 tokens = 16 batches
                prod = ring.tile([128, SC], f32, tag="prod")
                nc.vector.tensor_tensor(
                    out=prod[:].rearrange("p (b n) -> p b n", n=N),
                    in0=subgT[:, ch * SC:(ch + 1) * SC].rearrange(
                        "p (b n) -> p b n", n=N),
                    in1=u_sb[:, ch * 16:(ch + 1) * 16].rearrange(
                        "p (b one) -> p b one", one=1).to_broadcast([128, 16, N]),
                    op=Alu.mult)
                sc_sb = med.tile([1, SC], f32, tag="scsb")
                for (o0, o1) in ((0, 512), (512, SC)):
                    sc_ps = psC.tile([1, 512], f32, tag="sc", space="PSUM")
                    nc.tensor.matmul(out=sc_ps[:, :o1 - o0], lhsT=onescol[:],
                                     rhs=prod[:, o0:o1], start=True, stop=True)
                    nc.vector.tensor_copy(out=sc_sb[:, o0:o1],
                                          in_=sc_ps[:, :o1 - o0])
                nc.sync.dma_start(out=scores_dr[:, ch * SC:(ch + 1) * SC],
                                  in_=sc_sb[:])
            scoresb = med.tile([128, N], f32, tag="scoresb")
            nc.sync.dma_start(out=scoresb[:], in_=scores_dr[:])
            nc.vector.tensor_scalar_add(out=scoresb[:], in0=scoresb[:],
                                        scalar1=ct_sb[:, 0:1])
            nc.scalar.activation(out=scoresb[:], in_=scoresb[:], func=Act.Lrelu,
                                 alpha=0.01)
            esum = tiny.tile([128, 1], f32, tag="esum")
            nc.scalar.activation(out=scoresb[:], in_=scoresb[:], func=Act.Exp,
                                 accum_out=esum[:, 0:1])
            einv = tiny.tile([128, 1], f32, tag="einv")
            nc.vector.reciprocal(out=einv[:], in_=esum[:])
            nc.vector.tensor_scalar_mul(out=scoresb[:], in0=scoresb[:],
                                        scalar1=einv[:, 0:1])

            # att weights transposed [50, 128]; fold into adjts columns
            att_ps = psB.tile([N, 128], f32, tag="tr", space="PSUM")
            nc.tensor.matmul(out=att_ps[:], lhsT=scoresb[:], rhs=ident[:],
                             is_transpose=True, start=True, stop=True)
            atts = med.tile([N, 128], f32, tag="atts")
            nc.vector.tensor_copy(out=atts[:], in_=att_ps[:])
            # pair scales [115, 64]: rows 0..49 = att(even b), 64..113 =
            # att(odd b); rows 50/114 (sum_hrt tokens) stay 1.0
            attsp = med.tile([115, BL // 2], f32, tag="attsp")
            nc.vector.memset(attsp[:], 1.0)
            attev = atts[:].rearrange("p (j two) -> p j two", two=2)
            nc.vector.tensor_copy(
                out=attsp[0:50, :].rearrange("p (j one) -> p j one", one=1),
                in_=attev[:, :, 0:1])
            nc.vector.tensor_copy(
                out=attsp[64:114, :].rearrange("p (j one) -> p j one", one=1),
                in_=attev[:, :, 1:2])
            nc.vector.tensor_tensor(
                out=adjts_sb[:].rearrange("p (j i) -> p j i", j=BL // 2),
                in0=adjts_sb[:].rearrange("p (j i) -> p j i", j=BL // 2),
                in1=attsp[:].rearrange("p (j one) -> p j one", one=1
                                       ).to_broadcast([115, BL // 2, 115]),
                op=Alu.mult)

            # ---- Z = gcn_W @ [S | subgT] + gcn_b, laid out 51 tokens/b ----
            z51 = big.tile([128, BL * 51], f32, tag="bb")  # reuses nebg slot
            z51v = z51[:].rearrange("p (b c) -> p b c", c=51)
            zs_ps = psA.tile([128, 128], f32, tag="mm", space="PSUM")
            nc.tensor.matmul(out=zs_ps[:], lhsT=gcnwt_sb[:], rhs=s_sb[:],
                             start=True, stop=True)
            nc.scalar.activation(
                out=z51v[:, :, 50:51],
                in_=zs_ps[:].rearrange("p (b one) -> p b one", one=1),
                func=Act.Identity, bias=gcnb_sb[:, 0:1], scale=1.0)
            for ch in range(NSC):          # 800-token chunks = 16 batches
                z_ps = psA.tile([128, SC], f32, tag="mm", space="PSUM")
                nc.tensor.matmul(out=z_ps[:, 0:512], lhsT=gcnwth_sb[:],
                                 rhs=subgT[:, ch * SC:ch * SC + 512],
                                 start=True, stop=True)
                nc.tensor.matmul(out=z_ps[:, 512:SC], lhsT=gcnwth_sb[:],
                                 rhs=subgT[:, ch * SC + 512:(ch + 1) * SC],
                                 start=True, stop=True)
                nc.scalar.activation(
                    out=z51v[:, ch * 16:(ch + 1) * 16, 0:50],
                    in_=z_ps[:].rearrange("p (b n) -> p b n", n=N),
                    func=Act.Identity, bias=gcnb_sb[:, 0:1], scale=1.0)

            # ---- bmm1 + relu + stats (pairs at partitions 0 / 64) ----
            y1 = big.tile([128, (BL // 2) * 128], f32, tag="y1")
            sum1 = tiny.tile([128, 16], f32, tag="sum1")
            sum2 = tiny.tile([128, 16], f32, tag="sum2")
            for g8 in range(BL // 8):      # 4 pairs per iteration
                ztq_ev = psB.tile([51, 512], f32, tag="tr", space="PSUM")
                ztq_od = psB.tile([51, 512], f32, tag="tr", space="PSUM")
                for jj in range(4):
                    j = g8 * 4 + jj
                    nc.tensor.matmul(
                        out=ztq_ev[:, jj * 128:(jj + 1) * 128],
                        lhsT=z51[:, (2 * j) * 51:(2 * j + 1) * 51],
                        rhs=ident[:], is_transpose=True, start=True, stop=True)
                    nc.tensor.matmul(
                        out=ztq_od[:, jj * 128:(jj + 1) * 128],
                        lhsT=z51[:, (2 * j + 1) * 51:(2 * j + 2) * 51],
                        rhs=ident[:], is_transpose=True, start=True, stop=True)
                zt_sb = ring.tile([128, 512], f32, tag="zt")
                nc.vector.memset(zt_sb[:], 0.0)
                nc.vector.tensor_copy(out=zt_sb[0:51, :], in_=ztq_ev[:])
                nc.vector.tensor_copy(out=zt_sb[64:115, :], in_=ztq_od[:])
                y1_ps = psA.tile([128, 512], f32, tag="mm", space="PSUM")
                for jj in range(4):
                    j = g8 * 4 + jj
                    nc.tensor.matmul(
                        out=y1_ps[0:115, jj * 128:(jj + 1) * 128],
                        lhsT=adjts_sb[:, j * 115:(j + 1) * 115],
                        rhs=zt_sb[0:115, jj * 128:(jj + 1) * 128],
                        start=True, stop=True)
                nc.scalar.activation(out=y1[:, g8 * 512:(g8 + 1) * 512],
                                     in_=y1_ps[:], func=Act.Relu,
                                     accum_out=sum1[:, g8:g8 + 1])
                sqbuf = ring.tile([128, 512], f32, tag="sq")
                nc.scalar.activation(out=sqbuf[:],
                                     in_=y1[:, g8 * 512:(g8 + 1) * 512],
                                     func=Act.Square,
                                     accum_out=sum2[:, g8:g8 + 1])

            # ---- BN1 stats fold + AllReduce ----
            Ax = mybir.AxisListType
            st = tiny.tile([128, 2], f32, tag="st")
            nc.vector.tensor_reduce(out=st[:, 0:1], in_=sum1[:], axis=Ax.X,
                                    op=Alu.add)
            nc.vector.tensor_reduce(out=st[:, 1:2], in_=sum2[:], axis=Ax.X,
                                    op=Alu.add)
            sthi = tiny.tile([51, 2], f32, tag="sthi")
            nc.vector.tensor_copy(out=sthi[:], in_=st[64:115, :])
            stf = tiny.tile([51, 2], f32, tag="stf")
            nc.vector.tensor_tensor(out=stf[:], in0=st[0:51, :],
                                    in1=sthi[:], op=Alu.add)
            bnc_in = dpool.tile([51, 2], f32)
            bnc_out = dpool.tile([51, 2], f32)
            nc.gpsimd.dma_start(out=bnc_in[:], in_=stf[:])
            nc.gpsimd.collective_compute(
                "AllReduce", Alu.add, replica_groups=[list(range(NC))],
                ins=[bnc_in.opt()], outs=[bnc_out.opt()])
            stg = tiny.tile([51, 2], f32, tag="stg")
            nc.gpsimd.dma_start(out=stg[:], in_=bnc_out[:])

            inv_n = 1.0 / (B * 128.0)
            mcol = tiny.tile([51, 1], f32, tag="mcol")
            nc.vector.tensor_scalar_mul(out=mcol[:], in0=stg[:, 0:1],
                                        scalar1=inv_n)
            m2 = tiny.tile([51, 1], f32, tag="m2")
            nc.vector.tensor_mul(m2[:], mcol[:], mcol[:])
            vcol = tiny.tile([51, 1], f32, tag="vcol")
            nc.vector.tensor_scalar_mul(out=vcol[:], in0=stg[:, 1:2],
                                        scalar1=inv_n)
            nc.vector.tensor_sub(vcol[:], vcol[:], m2[:])
            epst = tiny.tile([51, 1], f32, tag="epst")
            nc.vector.memset(epst[:], EPS)
            sd = tiny.tile([51, 1], f32, tag="sd")
            nc.scalar.activation(out=sd[:], in_=vcol[:], func=Act.Sqrt,
                                 bias=epst[:, 0:1])
            rstd = tiny.tile([51, 1], f32, tag="rstd")
            nc.vector.reciprocal(out=rstd[:], in_=sd[:])
            alpha = tiny.tile([51, 1], f32, tag="alpha")
            nc.vector.tensor_mul(alpha[:], gam_sb[:], rstd[:])
            ccol = tiny.tile([51, 1], f32, tag="ccol")
            nc.vector.tensor_mul(ccol[:], mcol[:], alpha[:])
            nc.vector.tensor_sub(ccol[:], bet_sb[:], ccol[:])
            alpha2 = tiny.tile([128, 1], f32, tag="alpha2")
            nc.vector.memset(alpha2[:], 0.0)
            nc.vector.tensor_copy(out=alpha2[0:51, :], in_=alpha[:])
            nc.vector.tensor_copy(out=alpha2[64:115, :], in_=alpha[:])
            ccol2 = tiny.tile([128, 1], f32, tag="ccol2")
            nc.vector.memset(ccol2[:], 0.0)
            nc.vector.tensor_copy(out=ccol2[0:51, :], in_=ccol[:])
            nc.vector.tensor_copy(out=ccol2[64:115, :], in_=ccol[:])

            nc.vector.tensor_scalar(y1[:], y1[:], alpha2[:, 0:1],
                                    ccol2[:, 0:1], Alu.mult, Alu.add)

            # ---- bmm2 row 0 (column-major out) + relu + stats2 ----
            y2t_ps = psB.tile([128, 128], f32, tag="tr", space="PSUM")
            for j in range(BL // 2):
                nc.tensor.matmul(
                    out=y2t_ps[:, 2 * j:2 * j + 2],
                    lhsT=y1[0:115, j * 128:(j + 1) * 128],
                    rhs=a0_sb[0:115, 2 * j:2 * j + 2],
                    start=True, stop=True)
            p1 = tiny.tile([128, 1], f32, tag="p1")
            y2t_sb = med.tile([128, 128], f32, tag="y2t")
            nc.scalar.activation(out=y2t_sb[:], in_=y2t_ps[:], func=Act.Relu,
                                 accum_out=p1[:, 0:1])
            p2 = tiny.tile([128, 1], f32, tag="p2")
            y2sq = med.tile([128, 128], f32, tag="y2sq")
            nc.scalar.activation(out=y2sq[:], in_=y2t_sb[:], func=Act.Square,
                                 accum_out=p2[:, 0:1])
            # transpose to token-major for output
            y2m_ps = psC.tile([128, 128], f32, tag="sc", space="PSUM")
            nc.tensor.matmul(out=y2m_ps[:], lhsT=y2t_sb[:], rhs=ident[:],
                             is_transpose=True, start=True, stop=True)
            y2m_sb = med.tile([128, 128], f32, tag="y2m")
            nc.vector.tensor_copy(out=y2m_sb[:], in_=y2m_ps[:])
            nc.sync.dma_start(out=y2_out[:], in_=y2m_sb[:])

            pc = tiny.tile([128, 2], f32, tag="pc")
            nc.vector.tensor_copy(out=pc[:, 0:1], in_=p1[:])
            nc.vector.tensor_copy(out=pc[:, 1:2], in_=p2[:])
            s2_ps = psC.tile([1, 2], f32, tag="sc", space="PSUM")
            nc.tensor.matmul(out=s2_ps[:], lhsT=onescol[:], rhs=pc[:],
                             start=True, stop=True)
            s2_sb = tiny.tile([1, 2], f32, tag="s2sb")
            nc.vector.tensor_copy(out=s2_sb[:], in_=s2_ps[:])
            nc.sync.dma_start(out=s2_out[:], in_=s2_sb[:])

    nc.finalize()
    return nc


def _prep_core(ci, hrt, neb, nebr, adj):
    b0, b1 = ci * BL, (ci + 1) * BL
    hrt_l = np.asarray(hrt[b0:b1], np.int64)
    neb_l = np.asarray(neb[b0:b1], np.int64)
    nebr_l = np.asarray(nebr[b0:b1], np.int64)
    adj_l = np.asarray(adj[b0:b1], np.float32)

    neb_flat = neb_l.reshape(T)
    neb_off = np.ascontiguousarray(neb_flat.reshape(NB_CALLS, 128).T).astype(np.int32)

    tok = np.repeat(np.arange(T, dtype=np.int64), K)
    rr = nebr_l.reshape(T * K)
    cnt = np.zeros((1024, T), np.float32)
    np.add.at(cnt, (rr, tok), 1.0)
    cnt *= (1.0 / 8.0)
    c2r = cnt.reshape(8, 128, NCHUNK, CH).transpose(2, 1, 0, 3).reshape(
        NCHUNK, 128, 8 * CH)
    c2r = np.ascontiguousarray(c2r).astype(ml_dtypes.float8_e4m3)

    # pair-blocked lhsT: [102, 64, 115]; K rows 0..50 = batch 2j channels,
    # 51..101 = batch 2j+1; M cols 0..50 / 64..114 = the two output bands
    AT = adj_l.transpose(0, 2, 1)                   # AT[b, k, i] = adj[b, i, k]
    ATr = np.concatenate([AT[:, 1:51, :], AT[:, 0:1, :]], axis=1)  # token order
    ap_ = np.zeros((115, BL // 2, 115), np.float32)
    ap_[0:51, :, 0:51] = ATr[0::2].transpose(1, 0, 2)
    ap_[64:115, :, 64:115] = ATr[1::2].transpose(1, 0, 2)
    adjts = np.ascontiguousarray(ap_.reshape(115, (BL // 2) * 115))
    a0 = np.zeros((128, BL), np.float32)
    a0[0:51, 0::2] = adj_l[0::2, 0, :].T
    a0[64:115, 1::2] = adj_l[1::2, 0, :].T

    return {
        "neb_off": neb_off,
        "h_off": hrt_l[:, 0:1].astype(np.int32).copy(),
        "t_off": hrt_l[:, 2:3].astype(np.int32).copy(),
        "r_off": hrt_l[:, 1:2].astype(np.int32).copy(),
        "c2": c2r,
        "adjts": adjts,
        "a0": a0,
    }


def kernel(hrt, neb, nebr, adj, ent_W, rel_W, att_W, att_b, gcn_W, gcn_b,
           bn_gamma, bn_beta):
    from concourse.bass_utils import run_bass_kernel_spmd

    if "nc" not in _cache:
        _cache["nc"] = _build_program()
    nc = _cache["nc"]

    ent_np = np.ascontiguousarray(np.asarray(ent_W, np.float32))
    rel_np = np.ascontiguousarray(np.asarray(rel_W, np.float32))
    relw16 = np.zeros((1024, D), ml_dtypes.bfloat16)
    relw16[:NUM_RELS] = rel_np.astype(ml_dtypes.bfloat16)
    shared = {
        "ent": ent_np,
        "rel": rel_np,
        "relw16": np.ascontiguousarray(relw16.reshape(8, 128, D)),
        "att_wt": np.ascontiguousarray(np.asarray(att_W, np.float32).T),
        "att_w": np.ascontiguousarray(np.asarray(att_W, np.float32)),
        "gcn_wt": np.ascontiguousarray(np.asarray(gcn_W, np.float32).T),
        "gcn_wth": np.ascontiguousarray(np.asarray(gcn_W, np.float32).T * 0.5),
        "att_b": np.asarray(att_b, np.float32).reshape(D, 1).copy(),
        "gcn_b": np.asarray(gcn_b, np.float32).reshape(D, 1).copy(),
        "gam": np.asarray(bn_gamma, np.float32).reshape(51, 1).copy(),
        "bet": np.asarray(bn_beta, np.float32).reshape(51, 1).copy(),
    }
    in_maps = []
    for ci in range(NC):
        m = _prep_core(ci, hrt, neb, nebr, adj)
        m.update(shared)
        in_maps.append(m)
    _cache["in_maps"] = in_maps

    res = run_bass_kernel_spmd(nc, in_maps, core_ids=list(range(NC)))
    outs = res.results

    h_em = np.concatenate([o["h_out"] for o in outs])[:, None, :]
    t_em = np.concatenate([o["t_out"] for o in outs])[:, None, :]
    r_em = np.concatenate([o["r_out"] for o in outs])[:, None, :]
    y2 = np.concatenate([o["y2_out"] for o in outs])
    s2 = np.sum([o["s2_out"] for o in outs], axis=0)[0]
    m0 = s2[0] / (B * D)
    v0 = s2[1] / (B * D) - m0 * m0
    g0 = np.float32(np.asarray(bn_gamma).reshape(-1)[0])
    be0 = np.float32(np.asarray(bn_beta).reshape(-1)[0])
    tri = ((y2 - m0) / np.sqrt(v0 + EPS) * g0 + be0).astype(np.float32)[:, None, :]
    return (h_em, t_em, r_em, tri)
